# revision 15
# baseline (speedup 1.0000x reference)
"""Trainium2 Bass kernel for nn_Attention_2302102471003 (sparse LAS-style attention).

Contract: kernel(**inputs) takes the FULL unsharded inputs (as produced by
reference.setup_inputs) and returns the full output tuple
(context [B, 512] fp32, masked_attention [B, T] fp32).

Strategy:
  - masked_attention == softmax restricted to t < len_b (the full-T softmax
    followed by mask + L1-renorm cancels algebraically), so all compute beyond
    each sample's length is skipped (the "sparse" part).
  - Data-parallel over batch: 64 samples -> 8 cores x 8 slots. Samples are
    sorted by length; rank-octile s becomes slot s (one sample per core), so
    every core executes the identical static program (slot lengths padded to
    the octile max, multiple of 256 tokens).
  - Host pre-transposes listener_output to [feat, tokens] per core and
    pre-casts matmul inputs to bf16 (or the fp32r grid with TRN_ATT_PREC=f32r).
    bf16 matmuls stream at 1 elem/cycle with the weight load pipelined;
    fp32/fp32r self-loading matmuls serialize a ~107ns weight load per matmul.

Device program per core (slot-major):
  keyT[256, W] = lrelu(WkT.T @ xT)         (PE + copy/max epilogue)
  e[tok]       = keyT.T @ q                (PE, psum column pairs, one
                                            accumulation group per slot bank)
  V[tok, 512]  = lrelu(xT.T @ WvT)         (PE + epilogue)
  p = exp(e + mask_bias); S = gpsimd all-reduce; p_hat = p/S
  att out      = PE-transpose(p_hat) -> DMA
  context      = sum_j p_hat[:,j].T @ V_j  (PE accumulate in psum)
"""

import os
import numpy as np
import ml_dtypes

import concourse.tile as tile
import concourse.bass_isa as bass_isa
from concourse import bacc, mybir
from concourse.bass_utils import run_bass_kernel_spmd

F32 = mybir.dt.float32
F32R = mybir.dt.float32r
BF16 = mybir.dt.bfloat16
AF = mybir.ActivationFunctionType
ALU = mybir.AluOpType

B, T = 64, 2000
LDIM, SDIM, KQV, CDIM = 512, 512, 256, 512
NCORES, SLOTS = 8, 8
KCH = LDIM // 128          # 4 contraction chunks
MCH = KQV // 128           # 2 key feature chunks
MAXNB = 16                 # max 128-token subtiles per slot (2048 tokens)
NEG_BIG = -1.0e9

PREC = os.environ.get("TRN_ATT_PREC", "bf16")  # "bf16" | "f32r"

# stashes for the test harness
LAST_RESULTS = None
LAST_IN_MAPS = None


def round_fp32r(x: np.ndarray) -> np.ndarray:
    """Round fp32 to the fp32r grid (11-bit mantissa, low 12 bits zeroed, RNE)."""
    u = np.ascontiguousarray(x, dtype=np.float32).view(np.uint32)
    low = u & np.uint32(0xFFF)
    base = u & ~np.uint32(0xFFF)
    half = np.uint32(0x800)
    rup = (low > half) | ((low == half) & (((u >> np.uint32(12)) & np.uint32(1)) == 1))
    return (base + np.where(rup, np.uint32(0x1000), np.uint32(0))).view(np.float32)


def _mm_cast(x: np.ndarray) -> np.ndarray:
    if PREC == "bf16":
        return np.ascontiguousarray(x, dtype=np.float32).astype(ml_dtypes.bfloat16)
    return round_fp32r(x)


_MMDT = BF16 if PREC == "bf16" else F32R
_PROGRAM_CACHE: dict = {}


def _build_program(slot_nb: tuple, tot_nb: int, has_bias: bool, bench_iters: int = 0,
                   ablate: str = ""):
    """Build the SPMD Bass/Tile program for the given per-slot subtile counts.

    bench_iters > 0 wraps the steady-state body in an on-device For_i loop
    (benchmark builds only; the graded path uses bench_iters=0).
    """
    nc = bacc.Bacc("TRN2", target_bir_lowering=False, debug=False,
                   enable_asserts=True, num_devices=NCORES)
    TOT = tot_nb * 128
    MMDT = _MMDT

    xT_in = nc.dram_tensor("xT", [KCH, 128, TOT], MMDT, kind="ExternalInput").ap()
    wkT_in = nc.dram_tensor("wkT", [KCH, 128, KQV], MMDT, kind="ExternalInput").ap()
    wvT_in = nc.dram_tensor("wvT", [KCH, 128, CDIM], MMDT, kind="ExternalInput").ap()
    wqT_in = nc.dram_tensor("wqT", [KCH, 128, KQV], MMDT, kind="ExternalInput").ap()
    sT_in = nc.dram_tensor("sT", [KCH, 128, SLOTS], MMDT, kind="ExternalInput").ap()
    id_in = nc.dram_tensor("ident", [128, 128], F32, kind="ExternalInput").ap()
    mb_in = nc.dram_tensor("maskb", [128, SLOTS * 2 * MAXNB], F32, kind="ExternalInput").ap()
    if has_bias:
        bk_in = nc.dram_tensor("bk2", [MCH, 128, 1], F32, kind="ExternalInput").ap()
        bq_in = nc.dram_tensor("bq2", [MCH, 128, 1], F32, kind="ExternalInput").ap()
        bv_in = nc.dram_tensor("bvb", [128, CDIM], F32, kind="ExternalInput").ap()
    att_out = nc.dram_tensor("att", [SLOTS, 2 * MAXNB, 128], F32, kind="ExternalOutput").ap()
    ctx_out = nc.dram_tensor("ctx", [SLOTS, CDIM], F32, kind="ExternalOutput").ap()

    with tile.TileContext(nc) as tc:
        with (
            tc.tile_pool(name="cst", bufs=1) as cst,
            tc.tile_pool(name="xp", bufs=3) as xp,
            tc.tile_pool(name="kp", bufs=2) as kp,
            tc.tile_pool(name="vp", bufs=34) as vp,
            tc.tile_pool(name="ep", bufs=2) as ep,
            tc.tile_pool(name="tp", bufs=4) as tp,
            tc.tile_pool(name="pk_", bufs=2, space="PSUM") as pk_pool,
            tc.tile_pool(name="pv_", bufs=2, space="PSUM") as pv_pool,
            tc.tile_pool(name="pe_", bufs=2, space="PSUM") as pe_pool,
            tc.tile_pool(name="pt_", bufs=1, space="PSUM") as pt_pool,
            tc.tile_pool(name="pc_", bufs=1, space="PSUM") as pc_pool,
        ):
            # ---- constants / weights ----
            wkT, wvT, wqT, sT = [], [], [], []
            for k in range(KCH):
                wk = cst.tile([128, KQV], MMDT, tag=f"wk{k}")
                nc.sync.dma_start(wk[:], wkT_in[k])
                wkT.append(wk)
                wv = cst.tile([128, CDIM], MMDT, tag=f"wv{k}")
                nc.sync.dma_start(wv[:], wvT_in[k])
                wvT.append(wv)
                wq = cst.tile([128, KQV], MMDT, tag=f"wq{k}")
                nc.sync.dma_start(wq[:], wqT_in[k])
                wqT.append(wq)
                st = cst.tile([128, SLOTS], MMDT, tag=f"st{k}")
                nc.sync.dma_start(st[:], sT_in[k])
                sT.append(st)
            ident = cst.tile([128, 128], F32, tag="ident")
            nc.sync.dma_start(ident[:], id_in[:])
            maskb = cst.tile([128, SLOTS * 2 * MAXNB], F32, tag="maskb")
            nc.sync.dma_start(maskb[:], mb_in[:])
            if has_bias:
                bk2, bq2 = [], []
                for m in range(MCH):
                    bkm = cst.tile([128, 1], F32, tag=f"bk{m}")
                    nc.sync.dma_start(bkm[:], bk_in[m])
                    bk2.append(bkm)
                    bqm = cst.tile([128, 1], F32, tag=f"bq{m}")
                    nc.sync.dma_start(bqm[:], bq_in[m])
                    bq2.append(bqm)
                bvb = cst.tile([128, CDIM], F32, tag="bvb")
                nc.sync.dma_start(bvb[:], bv_in[:])
            ctx_sb = cst.tile([1, SLOTS * CDIM], F32, tag="ctxsb")
            dummy_mm = cst.tile([128, 512], _MMDT, tag="dummy_mm")
            nc.vector.memset(dummy_mm[:].bitcast(F32) if _MMDT == F32R else dummy_mm[:], 0.25)
            dummy_sink = cst.tile([128, 512], _MMDT, tag="dummy_sink")
            if "noctx" in ablate:
                nc.vector.memset(ctx_sb[:], 0.0)

            def leaky_from_psum(psum_ap, out_ap, tmp_tag, width, bias_col=None, bias_tile=None,
                                copy_engine="act"):
                """out = leaky_relu(psum [+ bias]) = max(0.2x, x), cast to MMDT.

                One psum read (the copy), then a DVE scalar_tensor_tensor on
                SBUF. copy_engine picks ACT or DVE for the psum->sbuf move to
                balance engine load.
                """
                if "noepi" in ablate:
                    return
                cp = tp.tile([128, 512], F32, tag=tmp_tag)
                if bias_col is not None:
                    nc.vector.tensor_scalar(cp[:, :width], psum_ap, bias_col[:], None, op0=ALU.add)
                elif bias_tile is not None:
                    nc.vector.tensor_tensor(cp[:, :width], psum_ap, bias_tile[:, :width], op=ALU.add)
                elif copy_engine == "dve":
                    nc.vector.tensor_copy(cp[:, :width], psum_ap)
                else:
                    nc.scalar.copy(cp[:, :width], psum_ap)
                if "nostt" not in ablate:
                    nc.vector.scalar_tensor_tensor(out_ap, cp[:, :width], 0.2, cp[:, :width],
                                                   op0=ALU.mult, op1=ALU.max)

            # ---- QT: [256, SLOTS] = lrelu(WqT.T @ sT), stored as interleaved pairs ----
            qT2 = []
            for m in range(MCH):
                q2m = cst.tile([128, 2 * SLOTS], MMDT, tag=f"q2{m}")
                nc.vector.memset(q2m[:].bitcast(F32) if MMDT == F32R else q2m[:], 0.0)
                pq = pk_pool.tile([128, 512], F32, tag="pk")
                for k in range(KCH):
                    nc.tensor.matmul(pq[:, :SLOTS], wqT[k][:, m * 128:(m + 1) * 128], sT[k][:],
                                     start=(k == 0), stop=(k == KCH - 1))
                leaky_from_psum(pq[:, :SLOTS], q2m[:, 0:2 * SLOTS:2], "qtmp", SLOTS,
                                bias_col=(bq2[m] if has_bias else None))
                qT2.append(q2m)

            # ---- main loop over slots (optionally repeated for benchmarking) ----
            import contextlib
            _bench_stack = contextlib.ExitStack()
            if bench_iters:
                _bench_stack.enter_context(
                    tc.For_i(0, bench_iters, 1,
                             hint_engines=(mybir.EngineType.PE,
                                           mybir.EngineType.DVE,
                                           mybir.EngineType.Activation,
                                           mybir.EngineType.SP,
                                           mybir.EngineType.Pool)))
            def emit_epilogue(s, nb, pe_s, v_tiles):
                """Softmax over the masked region + attention/context outputs.

                Emitted AFTER the next slot's matmul phase so the PE queue has
                work while the softmax chain (DVE/ACT/GPSIMD) resolves.
                Output DMAs are issued from the scalar engine (which produces
                their data) so they never block the input-DMA queue.
                """
                nb2 = 2 * nb
                e_sb = ep.tile([128, 2 * MAXNB], F32, tag="esb")
                if "noe" in ablate:
                    nc.vector.tensor_copy(e_sb[:, :nb2], maskb[:, s * 2 * MAXNB: s * 2 * MAXNB + nb2])
                else:
                    nc.vector.tensor_tensor(e_sb[:, :nb2], pe_s[:, :nb2],
                                            maskb[:, s * 2 * MAXNB: s * 2 * MAXNB + nb2], op=ALU.add)
                p_sb = ep.tile([128, 2 * MAXNB], F32, tag="psb")
                acc = ep.tile([128, 1], F32, tag="acc")
                nc.scalar.activation(p_sb[:, :nb2], e_sb[:, :nb2], AF.Exp, accum_out=acc[:])
                s_all = ep.tile([128, 1], F32, tag="sall")
                nc.gpsimd.partition_all_reduce(s_all[:], acc[:], channels=128,
                                               reduce_op=bass_isa.ReduceOp.add)
                rinv = ep.tile([128, 1], F32, tag="rinv")
                nc.vector.reciprocal(rinv[:], s_all[:])
                p_hat = ep.tile([128, 2 * MAXNB], F32, tag="ph")
                nc.vector.tensor_scalar(p_hat[:, :nb2], p_sb[:, :nb2], rinv[:], None, op0=ALU.mult)
                p_hat_b = ep.tile([128, 2 * MAXNB], MMDT, tag="phb")
                nc.vector.tensor_copy(p_hat_b[:, :nb2], p_hat[:, :nb2])

                # attention output: transpose [128, nb2] -> [nb2, 128] (fp32, exact)
                ptr = pt_pool.tile([2 * MAXNB, 128], F32, tag="ptr")
                nc.tensor.transpose(ptr[:nb2, :], p_hat[:, :nb2], ident[:])
                t_sb = ep.tile([2 * MAXNB, 128], F32, tag="tsb")
                nc.scalar.copy(t_sb[:nb2, :], ptr[:nb2, :])
                nc.scalar.dma_start(att_out[s, :nb2, :], t_sb[:nb2, :])

                # context accumulate
                if "noctx" not in ablate:
                    pc = pc_pool.tile([1, CDIM], F32, tag="pc")
                    for j in range(nb):
                        nc.tensor.matmul(pc[:], p_hat_b[:, 2 * j:2 * j + 1], v_tiles[j][:],
                                         start=(j == 0), stop=(j == nb - 1))
                    nc.scalar.copy(ctx_sb[0:1, s * CDIM:(s + 1) * CDIM], pc[:])

            pending = None
            for s in range(SLOTS):
                nb = slot_nb[s]
                off = sum(slot_nb[:s]) * 128
                pe_s = pe_pool.tile([128, 2 * MAXNB], F32, tag="pe")
                v_tiles = []
                for g in range((nb + 3) // 4):
                    cnt = min(4, nb - 4 * g)
                    W = 128 * cnt
                    goff = off + g * 512
                    if "nodma" in ablate:
                        xbig = None
                        xsl = lambda k, a, b: dummy_mm[:, a:b]
                    else:
                        # one merged DMA for all 4 contraction chunks of the group
                        xbig = xp.tile([128, KCH * 512], MMDT, tag="xbig")
                        nc.sync.dma_start(
                            xbig[:].rearrange("p (k t) -> p k t", k=KCH)[:, :, :W],
                            xT_in[:, :, goff:goff + W].rearrange("k p t -> p k t"),
                        )
                        xsl = lambda k, a, b: xbig[:, k * 512 + a:k * 512 + b]
                    # keyT chunks
                    keyms = []
                    for m in range(MCH):
                        pk = pk_pool.tile([128, 512], F32, tag="pk")
                        for k in range(KCH):
                            nc.tensor.matmul(pk[:, :W], wkT[k][:, m * 128:(m + 1) * 128],
                                             xsl(k, 0, W), start=(k == 0), stop=(k == KCH - 1))
                        if "noepi" in ablate or "nostt" in ablate:
                            if "noepi" not in ablate:
                                leaky_from_psum(pk[:, :W], dummy_sink[:, :W], f"ktmp{m}", W,
                                                copy_engine="act")
                            keyms.append(dummy_mm)
                        else:
                            keym = kp.tile([128, 512], MMDT, tag=f"key{m}")
                            leaky_from_psum(pk[:, :W], keym[:, :W], f"ktmp{m}", W,
                                            bias_col=(bk2[m] if has_bias else None),
                                            copy_engine="act")
                            keyms.append(keym)
                    # V tiles with energy matmuls interleaved between V matmuls
                    for jl in range(cnt):
                        j = 4 * g + jl
                        pv = pv_pool.tile([128, 512], F32, tag="pv")
                        for k in range(KCH):
                            nc.tensor.matmul(pv[:], xsl(k, jl * 128, (jl + 1) * 128),
                                             wvT[k][:], start=(k == 0), stop=(k == KCH - 1))
                            if k >= KCH - MCH and "noe" not in ablate:
                                m = k - (KCH - MCH)
                                # one accumulation group spans the whole pe_s
                                # bank (start=True zeroes a 2KB zero-region, so
                                # only the very first e-matmul of the slot sets)
                                nc.tensor.matmul(pe_s[:, 2 * j:2 * j + 2],
                                                 keyms[m][:, jl * 128:(jl + 1) * 128],
                                                 qT2[m][:, 2 * s:2 * s + 2],
                                                 start=(j == 0 and m == 0),
                                                 stop=(j == nb - 1 and m == MCH - 1))
                        if "noepi" in ablate or "nostt" in ablate:
                            if "noepi" not in ablate:
                                leaky_from_psum(pv[:], dummy_sink[:], "vtmp", 512,
                                                copy_engine=("dve" if jl % 2 else "act"))
                            v_tiles.append(dummy_mm)
                        else:
                            vt = vp.tile([128, 512], MMDT, tag="v")
                            leaky_from_psum(pv[:], vt[:], "vtmp", 512,
                                            bias_tile=(bvb if has_bias else None),
                                            copy_engine=("dve" if jl % 2 else "act"))
                            v_tiles.append(vt)

                if pending is not None:
                    emit_epilogue(*pending)
                pending = (s, nb, pe_s, v_tiles)
            emit_epilogue(*pending)

            _bench_stack.close()
            nc.scalar.dma_start(ctx_out.rearrange("s c -> (s c)").unsqueeze(0), ctx_sb[:])

    nc.compile()
    return nc


def kernel(listener_output, decoder_state, lengths, Wq, bq, Wk, bk, Wv, bv):
    global LAST_RESULTS, LAST_IN_MAPS
    listener_output = np.asarray(listener_output, dtype=np.float32)
    decoder_state = np.asarray(decoder_state, dtype=np.float32)
    lengths = np.asarray(lengths).astype(np.int64)
    Wq = np.asarray(Wq, dtype=np.float32)
    Wk = np.asarray(Wk, dtype=np.float32)
    Wv = np.asarray(Wv, dtype=np.float32)
    bq = np.asarray(bq, dtype=np.float32)
    bk = np.asarray(bk, dtype=np.float32)
    bv = np.asarray(bv, dtype=np.float32)
    has_bias = bool(np.any(bq) or np.any(bk) or np.any(bv))

    # ---- assignment: sort by length desc; octile s -> slot s, one per core ----
    order = np.argsort(-lengths, kind="stable")
    assign = order.reshape(SLOTS, NCORES)  # [slot, core] -> sample index
    slot_nb = []
    for s in range(SLOTS):
        mx = int(lengths[assign[s]].max())
        nb = (mx + 127) // 128  # 128-token subtiles
        slot_nb.append(max(1, min(MAXNB, nb)))
    slot_nb = tuple(slot_nb)
    tot_nb = sum(slot_nb)
    TOT = tot_nb * 128

    # ---- host packing ----
    np_mm = ml_dtypes.bfloat16 if PREC == "bf16" else np.float32
    lo_r = _mm_cast(listener_output)  # [B, T, 512] in matmul dtype
    xT = np.zeros((NCORES, LDIM, TOT), np_mm)
    maskb = np.full((NCORES, 128, SLOTS * 2 * MAXNB), NEG_BIG, np.float32)
    sC = np.zeros((NCORES, SLOTS, SDIM), np.float32)
    tok_idx = np.arange(128)
    for s in range(SLOTS):
        off = sum(slot_nb[:s]) * 128
        for c in range(NCORES):
            b = assign[s, c]
            L = int(lengths[b])
            xT[c, :, off:off + L] = lo_r[b, :L].T
            sC[c, s] = decoder_state[b]
            for j in range(slot_nb[s]):
                valid = (j * 128 + tok_idx) < L
                maskb[c, tok_idx[valid], s * 2 * MAXNB + 2 * j] = 0.0

    wkT = _mm_cast(Wk.T).reshape(KCH, 128, KQV)
    wvT = _mm_cast(Wv.T).reshape(KCH, 128, CDIM)
    wqT = _mm_cast(Wq.T).reshape(KCH, 128, KQV)
    sT = np.stack([_mm_cast(sC[c].T).reshape(KCH, 128, SLOTS) for c in range(NCORES)])
    ident = np.eye(128, dtype=np.float32)
    xT = xT.reshape(NCORES, KCH, 128, TOT)

    key = (slot_nb, has_bias, PREC)
    if key not in _PROGRAM_CACHE:
        _PROGRAM_CACHE[key] = _build_program(slot_nb, tot_nb, has_bias)
    nc = _PROGRAM_CACHE[key]

    in_maps = []
    for c in range(NCORES):
        m = {
            "xT": xT[c],
            "wkT": wkT,
            "wvT": wvT,
            "wqT": wqT,
            "sT": sT[c],
            "ident": ident,
            "maskb": maskb[c],
        }
        if has_bias:
            m["bk2"] = bk.reshape(MCH, 128, 1).astype(np.float32)
            m["bq2"] = bq.reshape(MCH, 128, 1).astype(np.float32)
            m["bvb"] = np.broadcast_to(bv, (128, CDIM)).copy()
        in_maps.append(m)

    LAST_IN_MAPS = in_maps
    trace = bool(int(os.environ.get("TRN_ATT_TRACE", "0")))
    res = run_bass_kernel_spmd(nc, in_maps, core_ids=list(range(NCORES)), trace=trace)
    LAST_RESULTS = res

    # ---- unshard ----
    context = np.zeros((B, CDIM), np.float32)
    masked_attention = np.zeros((B, T), np.float32)
    for s in range(SLOTS):
        for c in range(NCORES):
            b = assign[s, c]
            L = int(lengths[b])
            r = res.results[c]
            context[b] = r["ctx"][s]
            flat = r["att"][s, 0::2, :].reshape(MAXNB * 128)
            masked_attention[b, :L] = flat[:L]
    return context, masked_attention


# revision 16
# speedup vs baseline: 1.0674x; 1.0674x over previous
"""Trainium2 Bass kernel for nn_Attention_2302102471003 (sparse LAS-style attention).

Contract: kernel(**inputs) takes the FULL unsharded inputs (as produced by
reference.setup_inputs) and returns the full output tuple
(context [B, 512] fp32, masked_attention [B, T] fp32).

Strategy:
  - masked_attention == softmax restricted to t < len_b (the full-T softmax
    followed by mask + L1-renorm cancels algebraically), so all compute beyond
    each sample's length is skipped (the "sparse" part).
  - Data-parallel over batch: 64 samples -> 8 cores x 8 slots. Samples are
    sorted by length; rank-octile s becomes slot s (one sample per core), so
    every core executes the identical static program (slot lengths padded to
    the octile max, multiple of 256 tokens).
  - Host pre-transposes listener_output to [feat, tokens] per core and
    pre-casts matmul inputs to bf16 (or the fp32r grid with TRN_ATT_PREC=f32r).
    bf16 matmuls stream at 1 elem/cycle with the weight load pipelined;
    fp32/fp32r self-loading matmuls serialize a ~107ns weight load per matmul.

Device program per core (slot-major):
  keyT[256, W] = lrelu(WkT.T @ xT)         (PE + copy/max epilogue)
  e[tok]       = keyT.T @ q                (PE, psum column pairs, one
                                            accumulation group per slot bank)
  V[tok, 512]  = lrelu(xT.T @ WvT)         (PE + epilogue)
  p = exp(e + mask_bias); S = gpsimd all-reduce; p_hat = p/S
  att out      = PE-transpose(p_hat) -> DMA
  context      = sum_j p_hat[:,j].T @ V_j  (PE accumulate in psum)
"""

import os
import numpy as np
import ml_dtypes

import concourse.tile as tile
import concourse.bass_isa as bass_isa
from concourse import bacc, mybir
from concourse.bass_utils import run_bass_kernel_spmd

F32 = mybir.dt.float32
F32R = mybir.dt.float32r
BF16 = mybir.dt.bfloat16
AF = mybir.ActivationFunctionType
ALU = mybir.AluOpType

B, T = 64, 2000
LDIM, SDIM, KQV, CDIM = 512, 512, 256, 512
NCORES, SLOTS = 8, 8
KCH = LDIM // 128          # 4 contraction chunks
MCH = KQV // 128           # 2 key feature chunks
MAXNB = 16                 # max 128-token subtiles per slot (2048 tokens)
NEG_BIG = -1.0e9

PREC = os.environ.get("TRN_ATT_PREC", "bf16")  # "bf16" | "f32r"

# stashes for the test harness
LAST_RESULTS = None
LAST_IN_MAPS = None


def round_fp32r(x: np.ndarray) -> np.ndarray:
    """Round fp32 to the fp32r grid (11-bit mantissa, low 12 bits zeroed, RNE)."""
    u = np.ascontiguousarray(x, dtype=np.float32).view(np.uint32)
    low = u & np.uint32(0xFFF)
    base = u & ~np.uint32(0xFFF)
    half = np.uint32(0x800)
    rup = (low > half) | ((low == half) & (((u >> np.uint32(12)) & np.uint32(1)) == 1))
    return (base + np.where(rup, np.uint32(0x1000), np.uint32(0))).view(np.float32)


def _mm_cast(x: np.ndarray) -> np.ndarray:
    if PREC == "bf16":
        return np.ascontiguousarray(x, dtype=np.float32).astype(ml_dtypes.bfloat16)
    return round_fp32r(x)


_MMDT = BF16 if PREC == "bf16" else F32R
_PROGRAM_CACHE: dict = {}


def _build_program(slot_nb: tuple, tot_nb: int, has_bias: bool, bench_iters: int = 0,
                   ablate: str = ""):
    """Build the SPMD Bass/Tile program for the given per-slot subtile counts.

    bench_iters > 0 wraps the steady-state body in an on-device For_i loop
    (benchmark builds only; the graded path uses bench_iters=0).
    """
    nc = bacc.Bacc("TRN2", target_bir_lowering=False, debug=False,
                   enable_asserts=True, num_devices=NCORES)
    TOT = tot_nb * 128
    MMDT = _MMDT

    xT_in = nc.dram_tensor("xT", [KCH, 128, TOT], MMDT, kind="ExternalInput").ap()
    wkT_in = nc.dram_tensor("wkT", [KCH, 128, KQV], MMDT, kind="ExternalInput").ap()
    wvT_in = nc.dram_tensor("wvT", [KCH, 128, CDIM], MMDT, kind="ExternalInput").ap()
    wqT_in = nc.dram_tensor("wqT", [KCH, 128, KQV], MMDT, kind="ExternalInput").ap()
    sT_in = nc.dram_tensor("sT", [KCH, 128, SLOTS], MMDT, kind="ExternalInput").ap()
    id_in = nc.dram_tensor("ident", [128, 128], F32, kind="ExternalInput").ap()
    mb_in = nc.dram_tensor("maskb", [128, SLOTS * 2 * MAXNB], F32, kind="ExternalInput").ap()
    if has_bias:
        bk_in = nc.dram_tensor("bk2", [MCH, 128, 1], F32, kind="ExternalInput").ap()
        bq_in = nc.dram_tensor("bq2", [MCH, 128, 1], F32, kind="ExternalInput").ap()
        bv_in = nc.dram_tensor("bvb", [128, CDIM], F32, kind="ExternalInput").ap()
    att_out = nc.dram_tensor("att", [SLOTS, 2 * MAXNB, 128], F32, kind="ExternalOutput").ap()
    ctx_out = nc.dram_tensor("ctx", [SLOTS, CDIM], F32, kind="ExternalOutput").ap()

    with tile.TileContext(nc) as tc:
        with (
            tc.tile_pool(name="cst", bufs=1) as cst,
            tc.tile_pool(name="xp", bufs=3) as xp,
            tc.tile_pool(name="kp", bufs=2) as kp,
            tc.tile_pool(name="vp", bufs=34) as vp,
            tc.tile_pool(name="ep", bufs=2) as ep,
            tc.tile_pool(name="tp", bufs=4) as tp,
            tc.tile_pool(name="pk_", bufs=2, space="PSUM") as pk_pool,
            tc.tile_pool(name="pv_", bufs=3, space="PSUM") as pv_pool,
            tc.tile_pool(name="pe_", bufs=2, space="PSUM") as pe_pool,
            tc.tile_pool(name="pt_", bufs=1, space="PSUM") as pt_pool,
        ):
            # ---- constants / weights ----
            wkT, wvT, wqT, sT = [], [], [], []
            for k in range(KCH):
                wk = cst.tile([128, KQV], MMDT, tag=f"wk{k}")
                nc.sync.dma_start(wk[:], wkT_in[k])
                wkT.append(wk)
                wv = cst.tile([128, CDIM], MMDT, tag=f"wv{k}")
                nc.sync.dma_start(wv[:], wvT_in[k])
                wvT.append(wv)
                wq = cst.tile([128, KQV], MMDT, tag=f"wq{k}")
                nc.sync.dma_start(wq[:], wqT_in[k])
                wqT.append(wq)
                st = cst.tile([128, SLOTS], MMDT, tag=f"st{k}")
                nc.sync.dma_start(st[:], sT_in[k])
                sT.append(st)
            ident = cst.tile([128, 128], F32, tag="ident")
            nc.sync.dma_start(ident[:], id_in[:])
            maskb = cst.tile([128, SLOTS * 2 * MAXNB], F32, tag="maskb")
            nc.sync.dma_start(maskb[:], mb_in[:])
            if has_bias:
                bk2, bq2 = [], []
                for m in range(MCH):
                    bkm = cst.tile([128, 1], F32, tag=f"bk{m}")
                    nc.sync.dma_start(bkm[:], bk_in[m])
                    bk2.append(bkm)
                    bqm = cst.tile([128, 1], F32, tag=f"bq{m}")
                    nc.sync.dma_start(bqm[:], bq_in[m])
                    bq2.append(bqm)
                bvb = cst.tile([128, CDIM], F32, tag="bvb")
                nc.sync.dma_start(bvb[:], bv_in[:])
            ctx_sb = cst.tile([1, SLOTS * CDIM], F32, tag="ctxsb")
            dummy_mm = cst.tile([128, 512], _MMDT, tag="dummy_mm")
            nc.vector.memset(dummy_mm[:].bitcast(F32) if _MMDT == F32R else dummy_mm[:], 0.25)
            dummy_sink = cst.tile([128, 512], _MMDT, tag="dummy_sink")
            if "noctx" in ablate:
                nc.vector.memset(ctx_sb[:], 0.0)

            def leaky_from_psum(psum_ap, out_ap, tmp_tag, width, bias_col=None, bias_tile=None,
                                copy_engine="act"):
                """out = leaky_relu(psum [+ bias]) = max(0.2x, x), cast to MMDT.

                One psum read (the copy), then a DVE scalar_tensor_tensor on
                SBUF. copy_engine picks ACT or DVE for the psum->sbuf move to
                balance engine load.
                """
                if "noepi" in ablate:
                    return
                cp = tp.tile([128, 512], F32, tag=tmp_tag)
                if bias_col is not None:
                    nc.vector.tensor_scalar(cp[:, :width], psum_ap, bias_col[:], None, op0=ALU.add)
                elif bias_tile is not None:
                    nc.vector.tensor_tensor(cp[:, :width], psum_ap, bias_tile[:, :width], op=ALU.add)
                elif copy_engine == "dve":
                    nc.vector.tensor_copy(cp[:, :width], psum_ap)
                else:
                    nc.scalar.copy(cp[:, :width], psum_ap)
                if "nostt" not in ablate:
                    nc.vector.scalar_tensor_tensor(out_ap, cp[:, :width], 0.2, cp[:, :width],
                                                   op0=ALU.mult, op1=ALU.max)

            # ---- QT: [256, SLOTS] = lrelu(WqT.T @ sT), stored as interleaved pairs ----
            qT2 = []
            for m in range(MCH):
                q2m = cst.tile([128, 2 * SLOTS], MMDT, tag=f"q2{m}")
                nc.vector.memset(q2m[:].bitcast(F32) if MMDT == F32R else q2m[:], 0.0)
                pq = pk_pool.tile([128, 512], F32, tag="pk")
                for k in range(KCH):
                    nc.tensor.matmul(pq[:, :SLOTS], wqT[k][:, m * 128:(m + 1) * 128], sT[k][:],
                                     start=(k == 0), stop=(k == KCH - 1))
                leaky_from_psum(pq[:, :SLOTS], q2m[:, 0:2 * SLOTS:2], "qtmp", SLOTS,
                                bias_col=(bq2[m] if has_bias else None))
                qT2.append(q2m)

            # ---- main loop over slots (optionally repeated for benchmarking) ----
            import contextlib
            _bench_stack = contextlib.ExitStack()
            if bench_iters:
                _bench_stack.enter_context(
                    tc.For_i(0, bench_iters, 1,
                             hint_engines=(mybir.EngineType.PE,
                                           mybir.EngineType.DVE,
                                           mybir.EngineType.Activation,
                                           mybir.EngineType.SP,
                                           mybir.EngineType.Pool)))
            def emit_epilogue(s, nb, pe_s, v_tiles):
                """Softmax over the masked region + attention/context outputs.

                Emitted AFTER the next slot's matmul phase so the PE queue has
                work while the softmax chain (DVE/ACT/GPSIMD) resolves.
                Output DMAs are issued from the scalar engine (which produces
                their data) so they never block the input-DMA queue.
                """
                nb2 = 2 * nb
                e_sb = ep.tile([128, 2 * MAXNB], F32, tag="esb")
                if "noe" in ablate:
                    nc.vector.tensor_copy(e_sb[:, :nb2], maskb[:, s * 2 * MAXNB: s * 2 * MAXNB + nb2])
                else:
                    nc.vector.tensor_tensor(e_sb[:, :nb2], pe_s[:, :nb2],
                                            maskb[:, s * 2 * MAXNB: s * 2 * MAXNB + nb2], op=ALU.add)
                p_sb = ep.tile([128, 2 * MAXNB], F32, tag="psb")
                acc = ep.tile([128, 1], F32, tag="acc")
                nc.scalar.activation(p_sb[:, :nb2], e_sb[:, :nb2], AF.Exp, accum_out=acc[:])
                s_all = ep.tile([128, 1], F32, tag="sall")
                nc.gpsimd.partition_all_reduce(s_all[:], acc[:], channels=128,
                                               reduce_op=bass_isa.ReduceOp.add)
                rinv = ep.tile([128, 1], F32, tag="rinv")
                nc.vector.reciprocal(rinv[:], s_all[:])
                p_hat = ep.tile([128, 2 * MAXNB], F32, tag="ph")
                nc.vector.tensor_scalar(p_hat[:, :nb2], p_sb[:, :nb2], rinv[:], None, op0=ALU.mult)
                p_hat_b = ep.tile([128, 2 * MAXNB], MMDT, tag="phb")
                nc.vector.tensor_copy(p_hat_b[:, :nb2], p_hat[:, :nb2])

                # attention output: transpose [128, nb2] -> [nb2, 128] (fp32, exact)
                ptr = pt_pool.tile([2 * MAXNB, 128], F32, tag="ptc")
                nc.tensor.transpose(ptr[:nb2, :], p_hat[:, :nb2], ident[:])
                t_sb = ep.tile([2 * MAXNB, 128], F32, tag="tsb")
                nc.scalar.copy(t_sb[:nb2, :], ptr[:nb2, :])
                nc.scalar.dma_start(att_out[s, :nb2, :], t_sb[:nb2, :])

                # context accumulate
                if "noctx" not in ablate:
                    pc = pt_pool.tile([1, CDIM], F32, tag="ptc")
                    for j in range(nb):
                        nc.tensor.matmul(pc[:], p_hat_b[:, 2 * j:2 * j + 1], v_tiles[j][:],
                                         start=(j == 0), stop=(j == nb - 1))
                    nc.scalar.copy(ctx_sb[0:1, s * CDIM:(s + 1) * CDIM], pc[:])

            pending = None
            for s in range(SLOTS):
                nb = slot_nb[s]
                off = sum(slot_nb[:s]) * 128
                pe_s = pe_pool.tile([128, 2 * MAXNB], F32, tag="pe")
                v_tiles = []
                for g in range((nb + 3) // 4):
                    cnt = min(4, nb - 4 * g)
                    W = 128 * cnt
                    goff = off + g * 512
                    if "nodma" in ablate:
                        xbig = None
                        xsl = lambda k, a, b: dummy_mm[:, a:b]
                    else:
                        # one merged DMA for all 4 contraction chunks of the group
                        xbig = xp.tile([128, KCH * 512], MMDT, tag="xbig")
                        nc.sync.dma_start(
                            xbig[:].rearrange("p (k t) -> p k t", k=KCH)[:, :, :W],
                            xT_in[:, :, goff:goff + W].rearrange("k p t -> p k t"),
                        )
                        xsl = lambda k, a, b: xbig[:, k * 512 + a:k * 512 + b]
                    # keyT chunks
                    keyms = []
                    for m in range(MCH):
                        pk = pk_pool.tile([128, 512], F32, tag="pk")
                        for k in range(KCH):
                            nc.tensor.matmul(pk[:, :W], wkT[k][:, m * 128:(m + 1) * 128],
                                             xsl(k, 0, W), start=(k == 0), stop=(k == KCH - 1))
                        if "noepi" in ablate or "nostt" in ablate:
                            if "noepi" not in ablate:
                                leaky_from_psum(pk[:, :W], dummy_sink[:, :W], f"ktmp{m}", W,
                                                copy_engine="act")
                            keyms.append(dummy_mm)
                        else:
                            keym = kp.tile([128, 512], MMDT, tag=f"key{m}")
                            leaky_from_psum(pk[:, :W], keym[:, :W], f"ktmp{m}", W,
                                            bias_col=(bk2[m] if has_bias else None),
                                            copy_engine="act")
                            keyms.append(keym)
                    # V tiles with energy matmuls interleaved between V matmuls
                    for jl in range(cnt):
                        j = 4 * g + jl
                        pv = pv_pool.tile([128, 512], F32, tag="pv")
                        for k in range(KCH):
                            nc.tensor.matmul(pv[:], xsl(k, jl * 128, (jl + 1) * 128),
                                             wvT[k][:], start=(k == 0), stop=(k == KCH - 1))
                            if k >= KCH - MCH and "noe" not in ablate:
                                m = k - (KCH - MCH)
                                # one accumulation group spans the whole pe_s
                                # bank (start=True zeroes a 2KB zero-region, so
                                # only the very first e-matmul of the slot sets)
                                nc.tensor.matmul(pe_s[:, 2 * j:2 * j + 2],
                                                 keyms[m][:, jl * 128:(jl + 1) * 128],
                                                 qT2[m][:, 2 * s:2 * s + 2],
                                                 start=(j == 0 and m == 0),
                                                 stop=(j == nb - 1 and m == MCH - 1))
                        if "noepi" in ablate or "nostt" in ablate:
                            if "noepi" not in ablate:
                                leaky_from_psum(pv[:], dummy_sink[:], "vtmp", 512,
                                                copy_engine=("dve" if jl % 2 else "act"))
                            v_tiles.append(dummy_mm)
                        else:
                            vt = vp.tile([128, 512], MMDT, tag="v")
                            leaky_from_psum(pv[:], vt[:], "vtmp", 512,
                                            bias_tile=(bvb if has_bias else None),
                                            copy_engine=("dve" if jl % 2 else "act"))
                            v_tiles.append(vt)

                if pending is not None:
                    emit_epilogue(*pending)
                pending = (s, nb, pe_s, v_tiles)
            emit_epilogue(*pending)

            _bench_stack.close()
            nc.scalar.dma_start(ctx_out.rearrange("s c -> (s c)").unsqueeze(0), ctx_sb[:])

    nc.compile()
    return nc


def kernel(listener_output, decoder_state, lengths, Wq, bq, Wk, bk, Wv, bv):
    global LAST_RESULTS, LAST_IN_MAPS
    listener_output = np.asarray(listener_output, dtype=np.float32)
    decoder_state = np.asarray(decoder_state, dtype=np.float32)
    lengths = np.asarray(lengths).astype(np.int64)
    Wq = np.asarray(Wq, dtype=np.float32)
    Wk = np.asarray(Wk, dtype=np.float32)
    Wv = np.asarray(Wv, dtype=np.float32)
    bq = np.asarray(bq, dtype=np.float32)
    bk = np.asarray(bk, dtype=np.float32)
    bv = np.asarray(bv, dtype=np.float32)
    has_bias = bool(np.any(bq) or np.any(bk) or np.any(bv))

    # ---- assignment: sort by length desc; octile s -> slot s, one per core ----
    order = np.argsort(-lengths, kind="stable")
    assign = order.reshape(SLOTS, NCORES)  # [slot, core] -> sample index
    slot_nb = []
    for s in range(SLOTS):
        mx = int(lengths[assign[s]].max())
        nb = (mx + 127) // 128  # 128-token subtiles
        slot_nb.append(max(1, min(MAXNB, nb)))
    slot_nb = tuple(slot_nb)
    tot_nb = sum(slot_nb)
    TOT = tot_nb * 128

    # ---- host packing ----
    np_mm = ml_dtypes.bfloat16 if PREC == "bf16" else np.float32
    lo_r = _mm_cast(listener_output)  # [B, T, 512] in matmul dtype
    xT = np.zeros((NCORES, LDIM, TOT), np_mm)
    maskb = np.full((NCORES, 128, SLOTS * 2 * MAXNB), NEG_BIG, np.float32)
    sC = np.zeros((NCORES, SLOTS, SDIM), np.float32)
    tok_idx = np.arange(128)
    for s in range(SLOTS):
        off = sum(slot_nb[:s]) * 128
        for c in range(NCORES):
            b = assign[s, c]
            L = int(lengths[b])
            xT[c, :, off:off + L] = lo_r[b, :L].T
            sC[c, s] = decoder_state[b]
            for j in range(slot_nb[s]):
                valid = (j * 128 + tok_idx) < L
                maskb[c, tok_idx[valid], s * 2 * MAXNB + 2 * j] = 0.0

    wkT = _mm_cast(Wk.T).reshape(KCH, 128, KQV)
    wvT = _mm_cast(Wv.T).reshape(KCH, 128, CDIM)
    wqT = _mm_cast(Wq.T).reshape(KCH, 128, KQV)
    sT = np.stack([_mm_cast(sC[c].T).reshape(KCH, 128, SLOTS) for c in range(NCORES)])
    ident = np.eye(128, dtype=np.float32)
    xT = xT.reshape(NCORES, KCH, 128, TOT)

    key = (slot_nb, has_bias, PREC)
    if key not in _PROGRAM_CACHE:
        _PROGRAM_CACHE[key] = _build_program(slot_nb, tot_nb, has_bias)
    nc = _PROGRAM_CACHE[key]

    in_maps = []
    for c in range(NCORES):
        m = {
            "xT": xT[c],
            "wkT": wkT,
            "wvT": wvT,
            "wqT": wqT,
            "sT": sT[c],
            "ident": ident,
            "maskb": maskb[c],
        }
        if has_bias:
            m["bk2"] = bk.reshape(MCH, 128, 1).astype(np.float32)
            m["bq2"] = bq.reshape(MCH, 128, 1).astype(np.float32)
            m["bvb"] = np.broadcast_to(bv, (128, CDIM)).copy()
        in_maps.append(m)

    LAST_IN_MAPS = in_maps
    trace = bool(int(os.environ.get("TRN_ATT_TRACE", "0")))
    res = run_bass_kernel_spmd(nc, in_maps, core_ids=list(range(NCORES)), trace=trace)
    LAST_RESULTS = res

    # ---- unshard ----
    context = np.zeros((B, CDIM), np.float32)
    masked_attention = np.zeros((B, T), np.float32)
    for s in range(SLOTS):
        for c in range(NCORES):
            b = assign[s, c]
            L = int(lengths[b])
            r = res.results[c]
            context[b] = r["ctx"][s]
            flat = r["att"][s, 0::2, :].reshape(MAXNB * 128)
            masked_attention[b, :L] = flat[:L]
    return context, masked_attention


# revision 17
# speedup vs baseline: 1.1309x; 1.0595x over previous
"""Trainium2 Bass kernel for nn_Attention_2302102471003 (sparse LAS-style attention).

Contract: kernel(**inputs) takes the FULL unsharded inputs (as produced by
reference.setup_inputs) and returns the full output tuple
(context [B, 512] fp32, masked_attention [B, T] fp32).

Strategy:
  - masked_attention == softmax restricted to t < len_b (the full-T softmax
    followed by mask + L1-renorm cancels algebraically), so all compute beyond
    each sample's length is skipped (the "sparse" part).
  - Data-parallel over batch: 64 samples -> 8 cores x 8 slots. Samples are
    sorted by length; rank-octile s becomes slot s (one sample per core), so
    every core executes the identical static program (slot lengths padded to
    the octile max, multiple of 256 tokens).
  - Host pre-transposes listener_output to [feat, tokens] per core and
    pre-casts matmul inputs to bf16 (or the fp32r grid with TRN_ATT_PREC=f32r).
    bf16 matmuls stream at 1 elem/cycle with the weight load pipelined;
    fp32/fp32r self-loading matmuls serialize a ~107ns weight load per matmul.

Device program per core (slot-major):
  keyT[256, W] = lrelu(WkT.T @ xT)         (PE + copy/max epilogue)
  e[tok]       = keyT.T @ q                (PE, psum column pairs, one
                                            accumulation group per slot bank)
  V[tok, 512]  = lrelu(xT.T @ WvT)         (PE + epilogue)
  p = exp(e + mask_bias); S = gpsimd all-reduce; p_hat = p/S
  att out      = PE-transpose(p_hat) -> DMA
  context      = sum_j p_hat[:,j].T @ V_j  (PE accumulate in psum)
"""

import os
import numpy as np
import ml_dtypes

import concourse.tile as tile
import concourse.bass_isa as bass_isa
from concourse import bacc, mybir
from concourse.bass_utils import run_bass_kernel_spmd

F32 = mybir.dt.float32
F32R = mybir.dt.float32r
BF16 = mybir.dt.bfloat16
AF = mybir.ActivationFunctionType
ALU = mybir.AluOpType

B, T = 64, 2000
LDIM, SDIM, KQV, CDIM = 512, 512, 256, 512
NCORES, SLOTS = 8, 8
KCH = LDIM // 128          # 4 contraction chunks
MCH = KQV // 128           # 2 key feature chunks
MAXNB = 16                 # max 128-token subtiles per slot (2048 tokens)
NEG_BIG = -1.0e9

PREC = os.environ.get("TRN_ATT_PREC", "bf16")  # "bf16" | "f32r"

# stashes for the test harness
LAST_RESULTS = None
LAST_IN_MAPS = None


def round_fp32r(x: np.ndarray) -> np.ndarray:
    """Round fp32 to the fp32r grid (11-bit mantissa, low 12 bits zeroed, RNE)."""
    u = np.ascontiguousarray(x, dtype=np.float32).view(np.uint32)
    low = u & np.uint32(0xFFF)
    base = u & ~np.uint32(0xFFF)
    half = np.uint32(0x800)
    rup = (low > half) | ((low == half) & (((u >> np.uint32(12)) & np.uint32(1)) == 1))
    return (base + np.where(rup, np.uint32(0x1000), np.uint32(0))).view(np.float32)


def _mm_cast(x: np.ndarray) -> np.ndarray:
    if PREC == "bf16":
        return np.ascontiguousarray(x, dtype=np.float32).astype(ml_dtypes.bfloat16)
    return round_fp32r(x)


_MMDT = BF16 if PREC == "bf16" else F32R
_PROGRAM_CACHE: dict = {}


def _build_program(slot_nb: tuple, tot_nb: int, has_bias: bool, bench_iters: int = 0,
                   ablate: str = ""):
    """Build the SPMD Bass/Tile program for the given per-slot subtile counts.

    bench_iters > 0 wraps the steady-state body in an on-device For_i loop
    (benchmark builds only; the graded path uses bench_iters=0).
    """
    nc = bacc.Bacc("TRN2", target_bir_lowering=False, debug=False,
                   enable_asserts=True, num_devices=NCORES)
    TOT = tot_nb * 128
    MMDT = _MMDT

    xT_in = nc.dram_tensor("xT", [KCH, 128, TOT], MMDT, kind="ExternalInput").ap()
    wkT_in = nc.dram_tensor("wkT", [KCH, 128, KQV], MMDT, kind="ExternalInput").ap()
    wvT_in = nc.dram_tensor("wvT", [KCH, 128, CDIM], MMDT, kind="ExternalInput").ap()
    wqT_in = nc.dram_tensor("wqT", [KCH, 128, KQV], MMDT, kind="ExternalInput").ap()
    sT_in = nc.dram_tensor("sT", [KCH, 128, SLOTS], MMDT, kind="ExternalInput").ap()
    id_in = nc.dram_tensor("ident", [128, 128], F32, kind="ExternalInput").ap()
    mb_in = nc.dram_tensor("maskb", [128, SLOTS * 2 * MAXNB], F32, kind="ExternalInput").ap()
    if has_bias:
        bk_in = nc.dram_tensor("bk2", [MCH, 128, 1], F32, kind="ExternalInput").ap()
        bq_in = nc.dram_tensor("bq2", [MCH, 128, 1], F32, kind="ExternalInput").ap()
        bv_in = nc.dram_tensor("bvb", [128, CDIM], F32, kind="ExternalInput").ap()
    att_out = nc.dram_tensor("att", [SLOTS, 2 * MAXNB, 128], F32, kind="ExternalOutput").ap()
    ctx_out = nc.dram_tensor("ctx", [SLOTS, CDIM], F32, kind="ExternalOutput").ap()

    with tile.TileContext(nc) as tc:
        with (
            tc.tile_pool(name="cst", bufs=1) as cst,
            tc.tile_pool(name="xp", bufs=3) as xp,
            tc.tile_pool(name="kp", bufs=2) as kp,
            tc.tile_pool(name="vp", bufs=34) as vp,
            tc.tile_pool(name="ep", bufs=2) as ep,
            tc.tile_pool(name="tp", bufs=4) as tp,
            tc.tile_pool(name="pk_", bufs=2, space="PSUM") as pk_pool,
            tc.tile_pool(name="pv_", bufs=3, space="PSUM") as pv_pool,
            tc.tile_pool(name="pe_", bufs=2, space="PSUM") as pe_pool,
            tc.tile_pool(name="pt_", bufs=1, space="PSUM") as pt_pool,
        ):
            # ---- constants / weights ----
            wkT, wvT, wqT, sT = [], [], [], []
            for k in range(KCH):
                wk = cst.tile([128, KQV], MMDT, tag=f"wk{k}")
                nc.sync.dma_start(wk[:], wkT_in[k])
                wkT.append(wk)
                wv = cst.tile([128, CDIM], MMDT, tag=f"wv{k}")
                nc.sync.dma_start(wv[:], wvT_in[k])
                wvT.append(wv)
                wq = cst.tile([128, KQV], MMDT, tag=f"wq{k}")
                nc.sync.dma_start(wq[:], wqT_in[k])
                wqT.append(wq)
                st = cst.tile([128, SLOTS], MMDT, tag=f"st{k}")
                nc.sync.dma_start(st[:], sT_in[k])
                sT.append(st)
            ident = cst.tile([128, 128], F32, tag="ident")
            nc.sync.dma_start(ident[:], id_in[:])
            maskb = cst.tile([128, SLOTS * 2 * MAXNB], F32, tag="maskb")
            nc.sync.dma_start(maskb[:], mb_in[:])
            if has_bias:
                bk2, bq2 = [], []
                for m in range(MCH):
                    bkm = cst.tile([128, 1], F32, tag=f"bk{m}")
                    nc.sync.dma_start(bkm[:], bk_in[m])
                    bk2.append(bkm)
                    bqm = cst.tile([128, 1], F32, tag=f"bq{m}")
                    nc.sync.dma_start(bqm[:], bq_in[m])
                    bq2.append(bqm)
                bvb = cst.tile([128, CDIM], F32, tag="bvb")
                nc.sync.dma_start(bvb[:], bv_in[:])
            ctx_sb = cst.tile([1, SLOTS * CDIM], F32, tag="ctxsb")
            dummy_mm = cst.tile([128, 512], _MMDT, tag="dummy_mm")
            nc.vector.memset(dummy_mm[:].bitcast(F32) if _MMDT == F32R else dummy_mm[:], 0.25)
            dummy_sink = cst.tile([128, 512], _MMDT, tag="dummy_sink")
            if "noctx" in ablate:
                nc.vector.memset(ctx_sb[:], 0.0)

            def leaky_from_psum(psum_ap, out_ap, tmp_tag, width, bias_col=None, bias_tile=None,
                                copy_engine="act"):
                """out = leaky_relu(psum [+ bias]) = max(0.2x, x), cast to MMDT.

                One psum read (the copy), then a DVE scalar_tensor_tensor on
                SBUF. copy_engine picks ACT or DVE for the psum->sbuf move to
                balance engine load.
                """
                if "noepi" in ablate:
                    return
                # cp in MMDT (bf16): the 16-bit DVE path runs at 2x throughput;
                # DVE computes internally in fp32, so max(0.2x, x) is exact on
                # the rounded values (same error class as rounding after).
                cp = tp.tile([128, 512], MMDT, tag=tmp_tag)
                if bias_col is not None:
                    nc.vector.tensor_scalar(cp[:, :width], psum_ap, bias_col[:], None, op0=ALU.add)
                elif bias_tile is not None:
                    nc.vector.tensor_tensor(cp[:, :width], psum_ap, bias_tile[:, :width], op=ALU.add)
                elif copy_engine == "dve":
                    nc.vector.tensor_copy(cp[:, :width], psum_ap)
                else:
                    nc.scalar.copy(cp[:, :width], psum_ap)
                if "nostt" not in ablate:
                    nc.vector.scalar_tensor_tensor(out_ap, cp[:, :width], 0.2, cp[:, :width],
                                                   op0=ALU.mult, op1=ALU.max)

            # ---- QT: [256, SLOTS] = lrelu(WqT.T @ sT), stored as interleaved pairs ----
            qT2 = []
            for m in range(MCH):
                q2m = cst.tile([128, 2 * SLOTS], MMDT, tag=f"q2{m}")
                nc.vector.memset(q2m[:].bitcast(F32) if MMDT == F32R else q2m[:], 0.0)
                pq = pk_pool.tile([128, 512], F32, tag="pk")
                for k in range(KCH):
                    nc.tensor.matmul(pq[:, :SLOTS], wqT[k][:, m * 128:(m + 1) * 128], sT[k][:],
                                     start=(k == 0), stop=(k == KCH - 1))
                leaky_from_psum(pq[:, :SLOTS], q2m[:, 0:2 * SLOTS:2], "qtmp", SLOTS,
                                bias_col=(bq2[m] if has_bias else None))
                qT2.append(q2m)

            # ---- main loop over slots (optionally repeated for benchmarking) ----
            import contextlib
            _bench_stack = contextlib.ExitStack()
            if bench_iters:
                _bench_stack.enter_context(
                    tc.For_i(0, bench_iters, 1,
                             hint_engines=(mybir.EngineType.PE,
                                           mybir.EngineType.DVE,
                                           mybir.EngineType.Activation,
                                           mybir.EngineType.SP,
                                           mybir.EngineType.Pool)))
            def emit_epilogue(s, nb, pe_s, v_tiles):
                """Softmax over the masked region + attention/context outputs.

                Emitted AFTER the next slot's matmul phase so the PE queue has
                work while the softmax chain (DVE/ACT/GPSIMD) resolves.
                Output DMAs are issued from the scalar engine (which produces
                their data) so they never block the input-DMA queue.
                """
                nb2 = 2 * nb
                e_sb = ep.tile([128, 2 * MAXNB], F32, tag="esb")
                if "noe" in ablate:
                    nc.vector.tensor_copy(e_sb[:, :nb2], maskb[:, s * 2 * MAXNB: s * 2 * MAXNB + nb2])
                else:
                    nc.vector.tensor_tensor(e_sb[:, :nb2], pe_s[:, :nb2],
                                            maskb[:, s * 2 * MAXNB: s * 2 * MAXNB + nb2], op=ALU.add)
                p_sb = ep.tile([128, 2 * MAXNB], F32, tag="psb")
                acc = ep.tile([128, 1], F32, tag="acc")
                nc.scalar.activation(p_sb[:, :nb2], e_sb[:, :nb2], AF.Exp, accum_out=acc[:])
                s_all = ep.tile([128, 1], F32, tag="sall")
                nc.gpsimd.partition_all_reduce(s_all[:], acc[:], channels=128,
                                               reduce_op=bass_isa.ReduceOp.add)
                rinv = ep.tile([128, 1], F32, tag="rinv")
                nc.vector.reciprocal(rinv[:], s_all[:])
                p_hat = ep.tile([128, 2 * MAXNB], F32, tag="ph")
                nc.vector.tensor_scalar(p_hat[:, :nb2], p_sb[:, :nb2], rinv[:], None, op0=ALU.mult)
                p_hat_b = ep.tile([128, 2 * MAXNB], MMDT, tag="phb")
                nc.vector.tensor_copy(p_hat_b[:, :nb2], p_hat[:, :nb2])

                # attention output: transpose [128, nb2] -> [nb2, 128] (fp32, exact)
                ptr = pt_pool.tile([2 * MAXNB, 128], F32, tag="ptc")
                nc.tensor.transpose(ptr[:nb2, :], p_hat[:, :nb2], ident[:])
                t_sb = ep.tile([2 * MAXNB, 128], F32, tag="tsb")
                nc.scalar.copy(t_sb[:nb2, :], ptr[:nb2, :])
                nc.scalar.dma_start(att_out[s, :nb2, :], t_sb[:nb2, :])

                # context accumulate
                if "noctx" not in ablate:
                    pc = pt_pool.tile([1, CDIM], F32, tag="ptc")
                    for j in range(nb):
                        nc.tensor.matmul(pc[:], p_hat_b[:, 2 * j:2 * j + 1], v_tiles[j][:],
                                         start=(j == 0), stop=(j == nb - 1))
                    nc.scalar.copy(ctx_sb[0:1, s * CDIM:(s + 1) * CDIM], pc[:])

            pending = None
            for s in range(SLOTS):
                nb = slot_nb[s]
                off = sum(slot_nb[:s]) * 128
                pe_s = pe_pool.tile([128, 2 * MAXNB], F32, tag="pe")
                v_tiles = []
                for g in range((nb + 3) // 4):
                    cnt = min(4, nb - 4 * g)
                    W = 128 * cnt
                    goff = off + g * 512
                    if "nodma" in ablate:
                        xbig = None
                        xsl = lambda k, a, b: dummy_mm[:, a:b]
                    else:
                        # one merged DMA for all 4 contraction chunks of the group
                        xbig = xp.tile([128, KCH * 512], MMDT, tag="xbig")
                        nc.sync.dma_start(
                            xbig[:].rearrange("p (k t) -> p k t", k=KCH)[:, :, :W],
                            xT_in[:, :, goff:goff + W].rearrange("k p t -> p k t"),
                        )
                        xsl = lambda k, a, b: xbig[:, k * 512 + a:k * 512 + b]
                    # keyT chunks
                    keyms = []
                    for m in range(MCH):
                        pk = pk_pool.tile([128, 512], F32, tag="pk")
                        for k in range(KCH):
                            nc.tensor.matmul(pk[:, :W], wkT[k][:, m * 128:(m + 1) * 128],
                                             xsl(k, 0, W), start=(k == 0), stop=(k == KCH - 1))
                        if "noepi" in ablate or "nostt" in ablate:
                            if "noepi" not in ablate:
                                leaky_from_psum(pk[:, :W], dummy_sink[:, :W], f"ktmp{m}", W,
                                                copy_engine="act")
                            keyms.append(dummy_mm)
                        else:
                            keym = kp.tile([128, 512], MMDT, tag=f"key{m}")
                            leaky_from_psum(pk[:, :W], keym[:, :W], f"ktmp{m}", W,
                                            bias_col=(bk2[m] if has_bias else None),
                                            copy_engine="act")
                            keyms.append(keym)
                    # V tiles with energy matmuls interleaved between V matmuls
                    for jl in range(cnt):
                        j = 4 * g + jl
                        pv = pv_pool.tile([128, 512], F32, tag="pv")
                        for k in range(KCH):
                            nc.tensor.matmul(pv[:], xsl(k, jl * 128, (jl + 1) * 128),
                                             wvT[k][:], start=(k == 0), stop=(k == KCH - 1))
                            if k >= KCH - MCH and "noe" not in ablate:
                                m = k - (KCH - MCH)
                                # one accumulation group spans the whole pe_s
                                # bank (start=True zeroes a 2KB zero-region, so
                                # only the very first e-matmul of the slot sets)
                                nc.tensor.matmul(pe_s[:, 2 * j:2 * j + 2],
                                                 keyms[m][:, jl * 128:(jl + 1) * 128],
                                                 qT2[m][:, 2 * s:2 * s + 2],
                                                 start=(j == 0 and m == 0),
                                                 stop=(j == nb - 1 and m == MCH - 1))
                        if "noepi" in ablate or "nostt" in ablate:
                            if "noepi" not in ablate:
                                leaky_from_psum(pv[:], dummy_sink[:], "vtmp", 512,
                                                copy_engine=("dve" if jl % 2 else "act"))
                            v_tiles.append(dummy_mm)
                        else:
                            vt = vp.tile([128, 512], MMDT, tag="v")
                            leaky_from_psum(pv[:], vt[:], "vtmp", 512,
                                            bias_tile=(bvb if has_bias else None),
                                            copy_engine=("dve" if jl % 2 else "act"))
                            v_tiles.append(vt)

                if pending is not None:
                    emit_epilogue(*pending)
                pending = (s, nb, pe_s, v_tiles)
            emit_epilogue(*pending)

            _bench_stack.close()
            nc.scalar.dma_start(ctx_out.rearrange("s c -> (s c)").unsqueeze(0), ctx_sb[:])

    nc.compile()
    return nc


def kernel(listener_output, decoder_state, lengths, Wq, bq, Wk, bk, Wv, bv):
    global LAST_RESULTS, LAST_IN_MAPS
    listener_output = np.asarray(listener_output, dtype=np.float32)
    decoder_state = np.asarray(decoder_state, dtype=np.float32)
    lengths = np.asarray(lengths).astype(np.int64)
    Wq = np.asarray(Wq, dtype=np.float32)
    Wk = np.asarray(Wk, dtype=np.float32)
    Wv = np.asarray(Wv, dtype=np.float32)
    bq = np.asarray(bq, dtype=np.float32)
    bk = np.asarray(bk, dtype=np.float32)
    bv = np.asarray(bv, dtype=np.float32)
    has_bias = bool(np.any(bq) or np.any(bk) or np.any(bv))

    # ---- assignment: sort by length desc; octile s -> slot s, one per core ----
    order = np.argsort(-lengths, kind="stable")
    assign = order.reshape(SLOTS, NCORES)  # [slot, core] -> sample index
    slot_nb = []
    for s in range(SLOTS):
        mx = int(lengths[assign[s]].max())
        nb = (mx + 127) // 128  # 128-token subtiles
        slot_nb.append(max(1, min(MAXNB, nb)))
    slot_nb = tuple(slot_nb)
    tot_nb = sum(slot_nb)
    TOT = tot_nb * 128

    # ---- host packing ----
    np_mm = ml_dtypes.bfloat16 if PREC == "bf16" else np.float32
    lo_r = _mm_cast(listener_output)  # [B, T, 512] in matmul dtype
    xT = np.zeros((NCORES, LDIM, TOT), np_mm)
    maskb = np.full((NCORES, 128, SLOTS * 2 * MAXNB), NEG_BIG, np.float32)
    sC = np.zeros((NCORES, SLOTS, SDIM), np.float32)
    tok_idx = np.arange(128)
    for s in range(SLOTS):
        off = sum(slot_nb[:s]) * 128
        for c in range(NCORES):
            b = assign[s, c]
            L = int(lengths[b])
            xT[c, :, off:off + L] = lo_r[b, :L].T
            sC[c, s] = decoder_state[b]
            for j in range(slot_nb[s]):
                valid = (j * 128 + tok_idx) < L
                maskb[c, tok_idx[valid], s * 2 * MAXNB + 2 * j] = 0.0

    wkT = _mm_cast(Wk.T).reshape(KCH, 128, KQV)
    wvT = _mm_cast(Wv.T).reshape(KCH, 128, CDIM)
    wqT = _mm_cast(Wq.T).reshape(KCH, 128, KQV)
    sT = np.stack([_mm_cast(sC[c].T).reshape(KCH, 128, SLOTS) for c in range(NCORES)])
    ident = np.eye(128, dtype=np.float32)
    xT = xT.reshape(NCORES, KCH, 128, TOT)

    key = (slot_nb, has_bias, PREC)
    if key not in _PROGRAM_CACHE:
        _PROGRAM_CACHE[key] = _build_program(slot_nb, tot_nb, has_bias)
    nc = _PROGRAM_CACHE[key]

    in_maps = []
    for c in range(NCORES):
        m = {
            "xT": xT[c],
            "wkT": wkT,
            "wvT": wvT,
            "wqT": wqT,
            "sT": sT[c],
            "ident": ident,
            "maskb": maskb[c],
        }
        if has_bias:
            m["bk2"] = bk.reshape(MCH, 128, 1).astype(np.float32)
            m["bq2"] = bq.reshape(MCH, 128, 1).astype(np.float32)
            m["bvb"] = np.broadcast_to(bv, (128, CDIM)).copy()
        in_maps.append(m)

    LAST_IN_MAPS = in_maps
    trace = bool(int(os.environ.get("TRN_ATT_TRACE", "0")))
    res = run_bass_kernel_spmd(nc, in_maps, core_ids=list(range(NCORES)), trace=trace)
    LAST_RESULTS = res

    # ---- unshard ----
    context = np.zeros((B, CDIM), np.float32)
    masked_attention = np.zeros((B, T), np.float32)
    for s in range(SLOTS):
        for c in range(NCORES):
            b = assign[s, c]
            L = int(lengths[b])
            r = res.results[c]
            context[b] = r["ctx"][s]
            flat = r["att"][s, 0::2, :].reshape(MAXNB * 128)
            masked_attention[b, :L] = flat[:L]
    return context, masked_attention
